# revision 1
# baseline (speedup 1.0000x reference)
"""Trainium2 Bass kernel for nn_NPairsLoss (N-pairs loss over n=4096 rows).

Reference math (X = inputs.reshape(4096, 512), prod = X @ X.T, class/part row
masks): loss = (1/n) * sum_i [2*sum_{sadc_i} g_ij + sum_{dasc_i} g_ij] with
g_ij = log1p(S_i exp(-prod_ij)) and S_i the exp-sum over diff-class/diff-part
columns j.

Decomposition (exact to ~1e-7 relative):
    g_ij = ln(S_i) - prod_ij + e_ij/S_i,  e_ij = exp(prod_ij)
so every masked g-sum splits into count*ln(S), a masked *linear* prod sum
(= x_i . sum-vector, host GEMV), and a masked exp sum / S. The same-class
masked exp sums (Ec, Ecp) have support only on the ~32x32 per-class Gram
blocks (0.8% of the matrix) and are computed exactly on the host from tiny
per-class Grams of the same fp8-cast X the device multiplies.

DEVICE (8 cores, SPMD; rows sorted by (part, class), 512 rows/core, per-core
column order [own rows | rest of own part | other parts] so cols [0:1024) are
always the same-part region): the full n^2 work -
  fp8 DoubleRow GEMM (K=512 as [128 partitions x 4 subtiles], N=512 per
  matmul, fp32 PSUM) -> per 1024-col PSUM tile: ACT exp with fused fp32
  row-sum accum. Output per core: (128, 16) f32 = per (block k, tile q)
  row sums of exp; tile q=0 of each block is the same-part sum Ep.
HOST: S = Eall - Ep - Ec + Ecp, Ls = ln S, and the weighted assembly
    w = 2*(1024*Ls - Pq + Ep/S) + (4bc*Ls - Mp + Ec/S) - 3*(bc*Ls - Mpq + Ecp/S)
    loss = sum(w)/n   (bc = batch count of the row's class).
"""
import os
from contextlib import ExitStack

import numpy as np
import ml_dtypes

import concourse.bass as bass
import concourse.tile as tile
from concourse import bacc, mybir
from concourse import bass_utils

B, P, D, C = 1024, 4, 512, 128
N = B * P                      # 4096 rows
NCORES = 8
RPC = N // NCORES              # 512 rows per core
BLK = 128                      # rows per block (SBUF partitions)
NBLK = RPC // BLK              # 4 blocks per core
QUARTER = 2 * RPC              # 1024 cols = same-part region

FP8 = mybir.dt.float8e4
F32 = mybir.dt.float32
nfp8 = ml_dtypes.float8_e4m3fn

_CACHE = {}


def _build_nc():
    nc = bacc.Bacc(
        "TRN2",
        target_bir_lowering=False,
        debug=False,
        enable_asserts=False,
        num_devices=NCORES,
    )
    xt_d = nc.dram_tensor("xt", [BLK, 4 * N], FP8, kind="ExternalInput")
    zc_d = nc.dram_tensor("zc", [BLK, 1], F32, kind="ExternalInput")
    out_d = nc.dram_tensor("out", [BLK, 12], F32, kind="ExternalOutput")

    AF = mybir.ActivationFunctionType
    # psum tiles per block: [0:1024] (quarter/Ep), [1024:3072], [3072:4096]
    TILES = [(0, 1024, "edge"), (1024, 3072, "mid"), (3072, 4096, "edge")]

    with tile.TileContext(nc) as tc, ExitStack() as ctx:
        const = ctx.enter_context(tc.tile_pool(name="const", bufs=1))
        psum = ctx.enter_context(tc.tile_pool(name="psum", bufs=1, space="PSUM"))
        sink = ctx.enter_context(tc.tile_pool(name="sink", bufs=2))

        # zeros for the activation bias (DMA'd so no Pool/DVE memset is needed
        # and the exit barrier spans only SP/PE/ACT)
        zc_t = const.tile([BLK, 1], F32, tag="zc")
        nc.sync.dma_start(zc_t[:], zc_d.ap())
        # xt is X^T in fp8, stored as [128, s*4096 + c] with contraction index
        # d = s*128 + p (s = subtile 0..3, p = partition). DMA col-ascending,
        # fine-grained early pieces so the first GEMMs start ASAP.
        xtall = const.tile([BLK, 4 * N], FP8, tag="xtall")
        PIECES = [(0, 512), (512, 1024), (1024, 2048), (2048, 3072), (3072, 4096)]
        for plo, phi in PIECES:
            for s in range(4):
                lo = s * N + plo
                hi = s * N + phi
                nc.sync.dma_start(xtall[:, lo:hi], xt_d.ap()[:, lo:hi])
        xt3 = xtall[:].rearrange("p (s c) -> p s c", s=4)
        out_t = const.tile([BLK, 12], F32, tag="out")

        for k in range(NBLK):
            for q, (tlo, thi, tag) in enumerate(TILES):
                w = thi - tlo
                ps = psum.tile([BLK, w], F32, tag=tag, bufs=(2 if tag == "edge" else 1),
                               name=f"ps{k}_{q}")
                for sp in range(2):          # s-subtile pairs (DoubleRow K=256)
                    for half in range(w // 512):
                        j = (tlo // 512) + half
                        nc.tensor.matmul(
                            ps[:, half * 512:(half + 1) * 512],
                            xt3[:, 2 * sp:2 * sp + 2, k * BLK:(k + 1) * BLK],
                            xt3[:, 2 * sp:2 * sp + 2, j * 512:(j + 1) * 512],
                            start=(sp == 0),
                            stop=(sp == 1),
                            perf_mode=mybir.MatmulPerfMode.DoubleRow,
                        )
                # exp(prod) with fused fp32 row-sum -> out col; the bf16
                # exp values themselves are dead (sink tile)
                e_sink = sink.tile([BLK, w], mybir.dt.bfloat16, tag=f"es_{tag}",
                                   name=f"es{k}_{q}")
                nc.scalar.activation(
                    e_sink[:], ps[:], AF.Exp, bias=zc_t[:],
                    accum_out=out_t[:, 3 * k + q:3 * k + q + 1],
                )

        nc.sync.dma_start(out_d.ap(), out_t[:])

    nc.compile()
    return nc


def host_prep(inputs, targets):
    """Per-core device inputs + host-side aux for the combine step."""
    X = np.ascontiguousarray(np.asarray(inputs, dtype=np.float32).reshape(N, D))
    tg = np.asarray(targets).astype(np.int64)
    t = np.repeat(tg, P)
    part = np.tile(np.arange(P, dtype=np.int64), B)
    order = np.lexsort((t, part))
    X_s = X[order]
    t_s = t[order]
    X8 = X_s.astype(nfp8)
    in_maps = []
    for c in range(NCORES):
        p = c // 2
        own = np.arange(RPC * c, RPC * (c + 1))
        buddy = np.arange(RPC * (c ^ 1), RPC * ((c ^ 1) + 1))
        lo, hi = QUARTER * p, QUARTER * (p + 1)
        rest = np.concatenate([np.arange(0, lo), np.arange(hi, N)])
        colperm = np.concatenate([own, buddy, rest])
        # (D, N) -> (4, 128, N) -> (128, 4, N) -> (128, 4N); d = s*128 + p
        xtT = X8[colperm].T                                          # (D, N)
        xt = np.ascontiguousarray(
            xtT.reshape(4, BLK, N).transpose(1, 0, 2).reshape(BLK, 4 * N)
        )
        in_maps.append({"xt": xt, "zc": np.zeros((BLK, 1), np.float32)})
    aux = dict(Xb=X8.astype(np.float64), t_s=t_s, tg=tg)
    return in_maps, aux


def host_combine(outs, aux):
    Xb, t_s, tg = aux["Xb"], aux["t_s"], aux["tg"]
    part_s = np.repeat(np.arange(P), B)
    bc = np.bincount(tg, minlength=C)
    # linear prod sum vectors
    qsum = np.stack([Xb[part_s == p].sum(axis=0) for p in range(P)])
    onehot = np.zeros((N, C))
    onehot[np.arange(N), t_s] = 1.0
    clssum = onehot.T @ Xb
    cpsum = np.stack([onehot[part_s == p].T @ Xb[part_s == p] for p in range(P)])
    # exact same-class masked exp sums via per-class Grams (~32x32 each)
    Ec = np.zeros(N)
    Ecp = np.zeros(N)
    for c in range(C):
        rows_c = np.nonzero(t_s == c)[0]
        if len(rows_c) == 0:
            continue
        V = Xb[rows_c]
        E = np.exp(V @ V.T)
        Ec[rows_c] = E.sum(axis=1)
        pc = part_s[rows_c]
        for p in range(P):
            m = pc == p
            if m.any():
                Ecp[rows_c[m]] = E[np.ix_(m, m)].sum(axis=1)
    total = 0.0
    for ci, o in enumerate(outs):
        o = np.asarray(o, np.float64)
        rows = np.arange(RPC * ci, RPC * (ci + 1))
        p = ci // 2
        x = Xb[rows]
        cls = t_s[rows]
        Pq = x @ qsum[p]
        Mp = (x * clssum[cls]).sum(axis=1)
        Mpq = (x * cpsum[p][cls]).sum(axis=1)
        cnt_c = 4.0 * bc[cls]
        cnt_cp = 1.0 * bc[cls]
        for k in range(NBLK):
            r = slice(BLK * k, BLK * (k + 1))
            rr = rows[r]
            Ep = o[:, 3 * k + 0]
            Eall = o[:, 3 * k:3 * k + 3].sum(axis=1)
            S = Eall - Ep - Ec[rr] + Ecp[rr]
            Ls = np.log(S)
            Gp_ = 1024.0 * Ls - Pq[r] + Ep / S
            Gc_ = cnt_c[r] * Ls - Mp[r] + Ec[rr] / S
            Gcp_ = cnt_cp[r] * Ls - Mpq[r] + Ecp[rr] / S
            total += float((2.0 * Gp_ + Gc_ - 3.0 * Gcp_).sum())
    return np.float32(total / N)


def kernel(inputs, targets):
    if "nc" not in _CACHE:
        _CACHE["nc"] = _build_nc()
    nc = _CACHE["nc"]
    in_maps, aux = host_prep(inputs, targets)
    kwargs = {}
    if bool(int(os.environ.get("NPAIRS_TRACE", "0"))):
        kwargs = dict(trace=True, tmpdir=os.environ.get("NPAIRS_TMPDIR") or None)
    res = bass_utils.run_bass_kernel_spmd(
        nc, in_maps, core_ids=list(range(NCORES)), **kwargs
    )
    _CACHE["last_results"] = res
    outs = [r["out"] for r in res.results]
    return host_combine(outs, aux)



# revision 3
# speedup vs baseline: 1.5807x; 1.5807x over previous
"""Trainium2 Bass kernel for nn_NPairsLoss — symmetric half-Gram, v3.

See kernel_v2 docstring for the math/cover.  v3 engineering changes:
- 4 input DMAs with 3D access patterns (HWDGE fixed cost is 625ns/DMA,
  serialized — v2's 30 piece DMAs burned 19us of HWDGE).
- consts via Pool memset (no DMA); early 1-element Exp warms the ACT table
  off the critical path.
- vcol slots [L4 | R1 R2 R3 | R5 R6 | H]; main psum chunk order
  [diag | R1 | R2 | R3] so block 0 / block 3 tiles can split into
  diag-only + R-only activations (earlier start, shorter tail).
- colsums: DVE fp16 accumulate for blocks 0-2, PE ones-matmuls directly on
  block 3's and the stray's E tiles (PSUM-accumulated), halves split between
  ACT and DVE for the PSUM->SBUF copy.
"""
import os
from contextlib import ExitStack

import numpy as np
import ml_dtypes

import concourse.bass as bass
import concourse.tile as tile
from concourse import bacc, mybir
from concourse import bass_utils

B, P, D, C = 1024, 4, 512, 128
N = B * P
NCORES = 8
STRIP = 512
BLK = 128
VC = 3200   # L4[0:512) R1[512:1024) R2[1024:1536) R3[1536:2048) R5 R6 H[3072:3200)
FP8 = mybir.dt.float8e4
F16 = mybir.dt.float16
F32 = mybir.dt.float32
nfp8 = ml_dtypes.float8_e4m3fn

STRAYS = [(0, (2, 4)), (1, (3, 5))]
STRAY_PAIRS = {(0, 2), (0, 4), (1, 3), (1, 5)}

_CACHE = {}


def _solve_cover():
    edges = [(i, j) for i in range(8) for j in range(i + 1, 8)
             if (i, j) not in STRAY_PAIRS]
    assert len(edges) == 24
    out = {c: [] for c in range(8)}

    def bt(k):
        if k == len(edges):
            return True
        i, j = edges[k]
        for c, t in ((i, j), (j, i)):
            if len(out[c]) < 3:
                out[c].append(t)
                if bt(k + 1):
                    return True
                out[c].pop()
        return False

    assert bt(0)
    grids = {}
    for c in range(8):
        ts = out[c][:]
        p = c ^ 1
        if p in ts:
            ts.remove(p)
            ts = [p] + ts
        grids[c] = ts
    return grids


GRIDS = _solve_cover()

# colsum chunk -> (partition, col) slot in the pc PSUM tile
CSPOS = [(0, 0), (32, 0), (64, 0), (0, 512), (32, 512)]   # R1 R2 R3 R5 R6


def _build_nc():
    nc = bacc.Bacc(
        "TRN2",
        target_bir_lowering=False,
        debug=False,
        enable_asserts=False,
        num_devices=NCORES,
    )
    xt_d = nc.dram_tensor("xt", [BLK, 4 * VC], FP8, kind="ExternalInput")
    ro_d = nc.dram_tensor("ro", [BLK, 16], F32, kind="ExternalOutput")
    cs_d = nc.dram_tensor("cs", [65, 1024], F32, kind="ExternalOutput")

    AF = mybir.ActivationFunctionType
    DR = mybir.MatmulPerfMode.DoubleRow
    ADD = mybir.AluOpType.add
    AX = mybir.AxisListType.X

    with tile.TileContext(nc) as tc, ExitStack() as ctx:
        const = ctx.enter_context(tc.tile_pool(name="const", bufs=1))
        psum = ctx.enter_context(tc.tile_pool(name="psum", bufs=1, space="PSUM"))
        epool = ctx.enter_context(tc.tile_pool(name="e", bufs=2))

        zc_t = const.tile([BLK, 1], F32, tag="zc", name="zc_t")
        nc.gpsimd.memset(zc_t[:], 0.0)
        on_t = const.tile([BLK, 1], F16, tag="on", name="on_t")
        nc.gpsimd.memset(on_t[:], 1.0)
        warm = const.tile([BLK, 1], F16, tag="warm", name="warm")
        nc.scalar.activation(warm[:], zc_t[:], AF.Exp, bias=zc_t[:])

        xt = const.tile([BLK, 4 * VC], FP8, tag="xt", name="xt")
        xt3 = xt[:].rearrange("p (s c) -> p s c", s=4)
        xd3 = xt_d.ap().rearrange("p (s c) -> p s c", s=4)
        for lo, hi in ((0, 512), (512, 1024), (1024, 1536), (1536, 2048),
                       (2048, 3200)):
            nc.sync.dma_start(xt3[:, :, lo:hi], xd3[:, :, lo:hi])

        out_t = const.tile([BLK, 16], F32, tag="out", name="out_t")
        acc = const.tile([BLK, 1536], F16, tag="acc", name="acc")
        csb = const.tile([BLK, 1024], F32, tag="csb", name="csb")

        def main_mms(ps, b, gs=(0, 1, 2, 3)):
            lhs_lo = BLK * b
            for g in gs:             # g=0 diag (L4), g>=1 -> R_g
                rhs_lo = 0 if g == 0 else 512 * g
                for sp in range(2):
                    nc.tensor.matmul(
                        ps[:, 512 * g:512 * (g + 1)],
                        xt3[:, 2 * sp:2 * sp + 2, lhs_lo:lhs_lo + BLK],
                        xt3[:, 2 * sp:2 * sp + 2, rhs_lo:rhs_lo + 512],
                        start=(sp == 0), stop=(sp == 1), perf_mode=DR,
                    )

        def act(e_t, ps, lo, hi, col):
            nc.scalar.activation(e_t[:, lo:hi], ps[:, lo:hi], AF.Exp,
                                 bias=zc_t[:], accum_out=out_t[:, col:col + 1])

        def reduce2(e_t, b):
            nc.vector.tensor_reduce(out_t[:, 8 + 2 * b:9 + 2 * b],
                                    e_t[:, 0:512], axis=AX, op=ADD)
            nc.vector.tensor_reduce(out_t[:, 9 + 2 * b:10 + 2 * b],
                                    e_t[:, 512:1024], axis=AX, op=ADD)

        # blocks 0,1: diag chunk exp'd as soon as its matmuls land (only
        # needs the first DMA piece), R chunks follow per-piece
        e_tiles = {}
        ps01 = []
        for b in range(2):
            ps = psum.tile([BLK, 2048], F32, tag="m", bufs=2, name=f"ps{b}")
            ps01.append(ps)
            e_t = epool.tile([BLK, 2048], F16, tag="e", bufs=2, name=f"e{b}")
            e_tiles[b] = e_t
            main_mms(ps, b, gs=(0,))
            act(e_t, ps, 0, 512, 2 * b)            # d0 / d1
        for b in range(2):
            main_mms(ps01[b], b, gs=(1, 2, 3))
            act(e_tiles[b], ps01[b], 512, 2048, 2 * b + 1)   # R0 / R1
            if b == 0:
                nc.vector.tensor_copy(acc[:], e_tiles[0][:, 512:2048])
            else:
                nc.vector.tensor_add(acc[:], acc[:], e_tiles[1][:, 512:2048])
            reduce2(e_tiles[b], b)

        # block 2: full tile
        ps2 = psum.tile([BLK, 2048], F32, tag="m", bufs=2, name="ps2")
        main_mms(ps2, 2)
        e2 = epool.tile([BLK, 2048], F16, tag="e", bufs=2, name="e2")
        e_tiles[2] = e2
        act(e2, ps2, 0, 2048, 4)                   # b2
        nc.vector.tensor_add(acc[:], acc[:], e2[:, 512:2048])
        reduce2(e2, 2)

        # stray: mms -> exp (no DVE accumulation; PE-direct colsums)
        pss = psum.tile([BLK, 2048], F32, tag="m", bufs=2, name="pss")
        for g in range(2):
            rhs_lo = 2048 + 512 * g
            for sp in range(2):
                nc.tensor.matmul(
                    pss[:, 512 * g:512 * (g + 1)],
                    xt3[:, 2 * sp:2 * sp + 2, 3072:3200],
                    xt3[:, 2 * sp:2 * sp + 2, rhs_lo:rhs_lo + 512],
                    start=(sp == 0), stop=(sp == 1), perf_mode=DR,
                )
        e_s = epool.tile([BLK, 1024], F16, tag="es", bufs=1, name="es")
        act(e_s, pss, 0, 1024, 5)              # stray

        # block 3: R3 then d3 last (d3 has no colsum dependency)
        ps3 = psum.tile([BLK, 2048], F32, tag="m", bufs=2, name="ps3")
        main_mms(ps3, 3)
        e3 = epool.tile([BLK, 2048], F16, tag="e", bufs=2, name="e3")
        e_tiles[3] = e3
        act(e3, ps3, 512, 2048, 6)             # R3
        act(e3, ps3, 0, 512, 7)                # d3

        # colsums: chunks R1..R3 = acc(b0..b2) + e3 direct; R5,R6 = e_s direct
        pc = psum.tile([BLK, 2048], F32, tag="m", bufs=2, name="pc")
        for g in range(3, 5):
            p0, c0 = CSPOS[g]
            nc.tensor.matmul(pc[p0:p0 + 1, c0:c0 + 512], on_t[:],
                             e_s[:, 512 * (g - 3):512 * (g - 2)],
                             start=True, stop=True)
        for g in range(3):
            p0, c0 = CSPOS[g]
            nc.tensor.matmul(pc[p0:p0 + 1, c0:c0 + 512], on_t[:],
                             acc[:, 512 * g:512 * (g + 1)],
                             start=True, stop=False)
            nc.tensor.matmul(pc[p0:p0 + 1, c0:c0 + 512], on_t[:],
                             e3[:, 512 * (g + 1):512 * (g + 2)],
                             start=False, stop=True)
        # PSUM -> SBUF: right half (stray chunks, ready early) on Pool,
        # left half (main chunks) on DVE
        nc.scalar.copy(csb[0:65, 512:1024], pc[0:65, 512:1024])
        nc.vector.tensor_copy(csb[0:65, 0:512], pc[0:65, 0:512])
        reduce2(e3, 3)

        nc.sync.dma_start(ro_d.ap(), out_t[:])
        nc.sync.dma_start(cs_d.ap(), csb[0:65, :])

    nc.compile()
    return nc


def host_prep(inputs, targets):
    X = np.ascontiguousarray(np.asarray(inputs, dtype=np.float32).reshape(N, D))
    tg = np.asarray(targets).astype(np.int64)
    t = np.repeat(tg, P)
    part = np.tile(np.arange(P, dtype=np.int64), B)
    order = np.lexsort((t, part))
    X_s = X[order]
    t_s = t[order]
    X8 = X_s.astype(nfp8)
    in_maps = []
    for c in range(NCORES):
        t1, t2, t3 = GRIDS[c]
        a, (sb1, sb2) = STRAYS[0] if c < 4 else STRAYS[1]
        blk_i = c % 4
        cols = np.concatenate([
            np.arange(STRIP * c, STRIP * (c + 1)),
            np.arange(STRIP * t1, STRIP * (t1 + 1)),
            np.arange(STRIP * t2, STRIP * (t2 + 1)),
            np.arange(STRIP * t3, STRIP * (t3 + 1)),
            np.arange(STRIP * sb1, STRIP * (sb1 + 1)),
            np.arange(STRIP * sb2, STRIP * (sb2 + 1)),
            np.arange(STRIP * a + BLK * blk_i, STRIP * a + BLK * (blk_i + 1)),
        ])
        assert cols.shape[0] == VC
        xtT = X8[cols].T
        xt = np.ascontiguousarray(
            xtT.reshape(4, BLK, VC).transpose(1, 0, 2).reshape(BLK, 4 * VC))
        in_maps.append({"xt": xt})
    aux = dict(Xb=X8.astype(np.float64), t_s=t_s, tg=tg)
    return in_maps, aux


def host_combine(outs, aux):
    Xb, t_s, tg = aux["Xb"], aux["t_s"], aux["tg"]
    part_s = np.repeat(np.arange(P), B)
    bc = np.bincount(tg, minlength=C)

    Eall = np.zeros(N)
    Ep = np.zeros(N)
    # ro cols: 0 d0, 1 R0, 2 d1, 3 R1, 4 b2, 5 stray, 6 R3, 7 d3,
    #          8+2b diag-reduce, 9+2b R1-chunk-reduce
    ACOL = {0: (0, 1), 1: (2, 3), 2: (4,), 3: (6, 7)}
    for c in range(NCORES):
        ro = np.asarray(outs[c]["ro"], np.float64)
        csb = np.asarray(outs[c]["cs"], np.float64)
        cs = [csb[p0, c0:c0 + 512] for (p0, c0) in CSPOS]
        t1, t2, t3 = GRIDS[c]
        a, (sb1, sb2) = STRAYS[0] if c < 4 else STRAYS[1]
        blk_i = c % 4
        for b in range(4):
            rows = slice(STRIP * c + BLK * b, STRIP * c + BLK * (b + 1))
            for col in ACOL[b]:
                Eall[rows] += ro[:, col]
            Ep[rows] += ro[:, 8 + 2 * b]
            if t1 == (c ^ 1):
                Ep[rows] += ro[:, 9 + 2 * b]
        srows = slice(STRIP * a + BLK * blk_i, STRIP * a + BLK * (blk_i + 1))
        Eall[srows] += ro[:, 5]
        for g, tg_ in enumerate((t1, t2, t3)):
            rows = slice(STRIP * tg_, STRIP * (tg_ + 1))
            Eall[rows] += cs[g]
            if tg_ == (c ^ 1):
                Ep[rows] += cs[g]
        Eall[STRIP * sb1:STRIP * (sb1 + 1)] += cs[3]
        Eall[STRIP * sb2:STRIP * (sb2 + 1)] += cs[4]

    qsum = np.stack([Xb[part_s == p].sum(axis=0) for p in range(P)])
    onehot = np.zeros((N, C))
    onehot[np.arange(N), t_s] = 1.0
    clssum = onehot.T @ Xb
    cpsum = np.stack([onehot[part_s == p].T @ Xb[part_s == p] for p in range(P)])
    Ec = np.zeros(N)
    Ecp = np.zeros(N)
    for cl in range(C):
        rows_c = np.nonzero(t_s == cl)[0]
        if len(rows_c) == 0:
            continue
        V = Xb[rows_c]
        E = np.exp(V @ V.T)
        Ec[rows_c] = E.sum(axis=1)
        pc_ = part_s[rows_c]
        for p in range(P):
            m = pc_ == p
            if m.any():
                Ecp[rows_c[m]] = E[np.ix_(m, m)].sum(axis=1)

    Pq = (Xb * qsum[part_s]).sum(axis=1)
    Mp = (Xb * clssum[t_s]).sum(axis=1)
    Mpq = (Xb * cpsum[part_s, t_s]).sum(axis=1)
    cnt_c = 4.0 * bc[t_s]
    cnt_cp = 1.0 * bc[t_s]

    S = Eall - Ep - Ec + Ecp
    Ls = np.log(S)
    Gp = 1024.0 * Ls - Pq + Ep / S
    Gc = cnt_c * Ls - Mp + Ec / S
    Gcp = cnt_cp * Ls - Mpq + Ecp / S
    total = float((2.0 * Gp + Gc - 3.0 * Gcp).sum())
    return np.float32(total / N)


def kernel(inputs, targets):
    if "nc" not in _CACHE:
        _CACHE["nc"] = _build_nc()
    nc = _CACHE["nc"]
    in_maps, aux = host_prep(inputs, targets)
    res = bass_utils.run_bass_kernel_spmd(
        nc, in_maps, core_ids=list(range(NCORES)))
    _CACHE["last_results"] = res
    outs = [{"ro": r["ro"], "cs": r["cs"]} for r in res.results]
    return host_combine(outs, aux)


# revision 6
# speedup vs baseline: 1.6045x; 1.0151x over previous
"""Trainium2 Bass kernel for nn_NPairsLoss — symmetric half-Gram, v3.

See kernel_v2 docstring for the math/cover.  v3 engineering changes:
- 4 input DMAs with 3D access patterns (HWDGE fixed cost is 625ns/DMA,
  serialized — v2's 30 piece DMAs burned 19us of HWDGE).
- consts via Pool memset (no DMA); early 1-element Exp warms the ACT table
  off the critical path.
- vcol slots [L4 | R1 R2 R3 | R5 R6 | H]; main psum chunk order
  [diag | R1 | R2 | R3] so block 0 / block 3 tiles can split into
  diag-only + R-only activations (earlier start, shorter tail).
- colsums: DVE fp16 accumulate for blocks 0-2, PE ones-matmuls directly on
  block 3's and the stray's E tiles (PSUM-accumulated), halves split between
  ACT and DVE for the PSUM->SBUF copy.
"""
import os
from contextlib import ExitStack

import numpy as np
import ml_dtypes

import concourse.bass as bass
import concourse.tile as tile
from concourse import bacc, mybir
from concourse import bass_utils

B, P, D, C = 1024, 4, 512, 128
N = B * P
NCORES = 8
STRIP = 512
BLK = 128
VC = 3200   # L4[0:512) R1[512:1024) R2[1024:1536) R3[1536:2048) R5 R6 H[3072:3200)
FP8 = mybir.dt.float8e4
F16 = mybir.dt.float16
F32 = mybir.dt.float32
nfp8 = ml_dtypes.float8_e4m3fn

STRAYS = [(0, (2, 4)), (1, (3, 5))]
STRAY_PAIRS = {(0, 2), (0, 4), (1, 3), (1, 5)}

_CACHE = {}


def _solve_cover():
    edges = [(i, j) for i in range(8) for j in range(i + 1, 8)
             if (i, j) not in STRAY_PAIRS]
    assert len(edges) == 24
    out = {c: [] for c in range(8)}

    def bt(k):
        if k == len(edges):
            return True
        i, j = edges[k]
        for c, t in ((i, j), (j, i)):
            if len(out[c]) < 3:
                out[c].append(t)
                if bt(k + 1):
                    return True
                out[c].pop()
        return False

    assert bt(0)
    grids = {}
    for c in range(8):
        ts = out[c][:]
        p = c ^ 1
        if p in ts:
            ts.remove(p)
            ts = [p] + ts
        grids[c] = ts
    return grids


GRIDS = _solve_cover()

# colsum chunk -> (partition, col) slot in the pc PSUM tile
CSPOS = [(0, 0), (32, 0), (64, 0), (0, 512), (32, 512)]   # R1 R2 R3 R5 R6


def _build_nc():
    nc = bacc.Bacc(
        "TRN2",
        target_bir_lowering=False,
        debug=False,
        enable_asserts=False,
        num_devices=NCORES,
    )
    xt_d = nc.dram_tensor("xt", [BLK, 4 * VC], FP8, kind="ExternalInput")
    ro_d = nc.dram_tensor("ro", [BLK, 16], F32, kind="ExternalOutput")
    cs_d = nc.dram_tensor("cs", [65, 1024], F32, kind="ExternalOutput")

    AF = mybir.ActivationFunctionType
    DR = mybir.MatmulPerfMode.DoubleRow
    ADD = mybir.AluOpType.add
    AX = mybir.AxisListType.X

    with tile.TileContext(nc) as tc, ExitStack() as ctx:
        const = ctx.enter_context(tc.tile_pool(name="const", bufs=1))
        psum = ctx.enter_context(tc.tile_pool(name="psum", bufs=1, space="PSUM"))
        epool = ctx.enter_context(tc.tile_pool(name="e", bufs=2))

        zc_t = const.tile([BLK, 1], F32, tag="zc", name="zc_t")
        nc.gpsimd.memset(zc_t[:], 0.0)
        on_t = const.tile([BLK, 1], F16, tag="on", name="on_t")
        nc.gpsimd.memset(on_t[:], 1.0)
        warm = const.tile([BLK, 1], F16, tag="warm", name="warm")
        nc.scalar.activation(warm[:], zc_t[:], AF.Exp, bias=zc_t[:])

        xt = const.tile([BLK, 4 * VC], FP8, tag="xt", name="xt")
        xt3 = xt[:].rearrange("p (s c) -> p s c", s=4)
        xd3 = xt_d.ap().rearrange("p (s c) -> p s c", s=4)
        for lo, hi in ((0, 512), (512, 1024), (1024, 1536), (1536, 2048),
                       (2048, 3200)):
            nc.sync.dma_start(xt3[:, :, lo:hi], xd3[:, :, lo:hi])

        out_t = const.tile([BLK, 16], F32, tag="out", name="out_t")
        acc = const.tile([BLK, 1536], F16, tag="acc", name="acc")
        csb = const.tile([BLK, 1024], F32, tag="csb", name="csb")

        def main_mms(ps, b, gs=(0, 1, 2, 3)):
            lhs_lo = BLK * b
            for g in gs:             # g=0 diag (L4), g>=1 -> R_g
                rhs_lo = 0 if g == 0 else 512 * g
                for sp in range(2):
                    nc.tensor.matmul(
                        ps[:, 512 * g:512 * (g + 1)],
                        xt3[:, 2 * sp:2 * sp + 2, lhs_lo:lhs_lo + BLK],
                        xt3[:, 2 * sp:2 * sp + 2, rhs_lo:rhs_lo + 512],
                        start=(sp == 0), stop=(sp == 1), perf_mode=DR,
                    )

        def act(e_t, ps, lo, hi, col):
            nc.scalar.activation(e_t[:, lo:hi], ps[:, lo:hi], AF.Exp,
                                 bias=zc_t[:], accum_out=out_t[:, col:col + 1])

        def reduce2(e_t, b):
            nc.vector.tensor_reduce(out_t[:, 8 + 2 * b:9 + 2 * b],
                                    e_t[:, 0:512], axis=AX, op=ADD)
            nc.vector.tensor_reduce(out_t[:, 9 + 2 * b:10 + 2 * b],
                                    e_t[:, 512:1024], axis=AX, op=ADD)

        # blocks 0,1: diag chunk exp'd as soon as its matmuls land (only
        # needs the first DMA piece), R chunks follow per-piece
        e_tiles = {}
        ps01 = []
        for b in range(2):
            ps = psum.tile([BLK, 2048], F32, tag="m", bufs=2, name=f"ps{b}")
            ps01.append(ps)
            e_t = epool.tile([BLK, 2048], F16, tag="e", bufs=2, name=f"e{b}")
            e_tiles[b] = e_t
            main_mms(ps, b, gs=(0,))
            act(e_t, ps, 0, 512, 2 * b)            # d0 / d1
        for b in range(2):
            main_mms(ps01[b], b, gs=(1, 2, 3))
            act(e_tiles[b], ps01[b], 512, 2048, 2 * b + 1)   # R0 / R1
            if b == 0:
                nc.vector.tensor_copy(acc[:], e_tiles[0][:, 512:2048])
            else:
                nc.vector.tensor_add(acc[:], acc[:], e_tiles[1][:, 512:2048])
            reduce2(e_tiles[b], b)

        # block 2: full tile
        ps2 = psum.tile([BLK, 2048], F32, tag="m", bufs=2, name="ps2")
        main_mms(ps2, 2)
        e2 = epool.tile([BLK, 2048], F16, tag="e", bufs=2, name="e2")
        e_tiles[2] = e2
        act(e2, ps2, 0, 2048, 4)                   # b2
        nc.vector.tensor_add(acc[:], acc[:], e2[:, 512:2048])
        reduce2(e2, 2)

        # stray: mms -> exp (no DVE accumulation; PE-direct colsums)
        pss = psum.tile([BLK, 2048], F32, tag="m", bufs=2, name="pss")
        for g in range(2):
            rhs_lo = 2048 + 512 * g
            for sp in range(2):
                nc.tensor.matmul(
                    pss[:, 512 * g:512 * (g + 1)],
                    xt3[:, 2 * sp:2 * sp + 2, 3072:3200],
                    xt3[:, 2 * sp:2 * sp + 2, rhs_lo:rhs_lo + 512],
                    start=(sp == 0), stop=(sp == 1), perf_mode=DR,
                )
        e_s = epool.tile([BLK, 1024], F16, tag="es", bufs=1, name="es")
        act(e_s, pss, 0, 1024, 5)              # stray

        # block 3: R3 then d3 last (d3 has no colsum dependency)
        ps3 = psum.tile([BLK, 2048], F32, tag="m", bufs=2, name="ps3")
        main_mms(ps3, 3)
        e3 = epool.tile([BLK, 2048], F16, tag="e", bufs=2, name="e3")
        e_tiles[3] = e3
        act(e3, ps3, 512, 2048, 6)             # R3
        act(e3, ps3, 0, 512, 7)                # d3

        # colsums: chunks R1..R3 = acc(b0..b2) + e3 direct; R5,R6 = e_s direct
        pc = psum.tile([BLK, 2048], F32, tag="m", bufs=2, name="pc")
        for g in range(3, 5):
            p0, c0 = CSPOS[g]
            nc.tensor.matmul(pc[p0:p0 + 1, c0:c0 + 512], on_t[:],
                             e_s[:, 512 * (g - 3):512 * (g - 2)],
                             start=True, stop=True)
        for g in range(3):
            p0, c0 = CSPOS[g]
            nc.tensor.matmul(pc[p0:p0 + 1, c0:c0 + 512], on_t[:],
                             acc[:, 512 * g:512 * (g + 1)],
                             start=True, stop=False)
            nc.tensor.matmul(pc[p0:p0 + 1, c0:c0 + 512], on_t[:],
                             e3[:, 512 * (g + 1):512 * (g + 2)],
                             start=False, stop=True)
        # PSUM -> SBUF: right half (stray chunks, ready early) on Pool,
        # left half (main chunks) on DVE
        nc.scalar.copy(csb[0:65, 512:1024], pc[0:65, 512:1024])
        nc.vector.tensor_copy(csb[0:65, 0:512], pc[0:65, 0:512])
        reduce2(e3, 3)

        nc.sync.dma_start(cs_d.ap()[:, 512:1024], csb[0:65, 512:1024])
        nc.sync.dma_start(ro_d.ap(), out_t[:])
        nc.sync.dma_start(cs_d.ap()[:, 0:512], csb[0:65, 0:512])

    nc.compile()
    return nc


def host_prep(inputs, targets):
    X = np.ascontiguousarray(np.asarray(inputs, dtype=np.float32).reshape(N, D))
    tg = np.asarray(targets).astype(np.int64)
    t = np.repeat(tg, P)
    part = np.tile(np.arange(P, dtype=np.int64), B)
    order = np.lexsort((t, part))
    X_s = X[order]
    t_s = t[order]
    X8 = X_s.astype(nfp8)
    in_maps = []
    for c in range(NCORES):
        t1, t2, t3 = GRIDS[c]
        a, (sb1, sb2) = STRAYS[0] if c < 4 else STRAYS[1]
        blk_i = c % 4
        cols = np.concatenate([
            np.arange(STRIP * c, STRIP * (c + 1)),
            np.arange(STRIP * t1, STRIP * (t1 + 1)),
            np.arange(STRIP * t2, STRIP * (t2 + 1)),
            np.arange(STRIP * t3, STRIP * (t3 + 1)),
            np.arange(STRIP * sb1, STRIP * (sb1 + 1)),
            np.arange(STRIP * sb2, STRIP * (sb2 + 1)),
            np.arange(STRIP * a + BLK * blk_i, STRIP * a + BLK * (blk_i + 1)),
        ])
        assert cols.shape[0] == VC
        xtT = X8[cols].T
        xt = np.ascontiguousarray(
            xtT.reshape(4, BLK, VC).transpose(1, 0, 2).reshape(BLK, 4 * VC))
        in_maps.append({"xt": xt})
    aux = dict(Xb=X8.astype(np.float64), t_s=t_s, tg=tg)
    return in_maps, aux


def host_combine(outs, aux):
    Xb, t_s, tg = aux["Xb"], aux["t_s"], aux["tg"]
    part_s = np.repeat(np.arange(P), B)
    bc = np.bincount(tg, minlength=C)

    Eall = np.zeros(N)
    Ep = np.zeros(N)
    # ro cols: 0 d0, 1 R0, 2 d1, 3 R1, 4 b2, 5 stray, 6 R3, 7 d3,
    #          8+2b diag-reduce, 9+2b R1-chunk-reduce
    ACOL = {0: (0, 1), 1: (2, 3), 2: (4,), 3: (6, 7)}
    for c in range(NCORES):
        ro = np.asarray(outs[c]["ro"], np.float64)
        csb = np.asarray(outs[c]["cs"], np.float64)
        cs = [csb[p0, c0:c0 + 512] for (p0, c0) in CSPOS]
        t1, t2, t3 = GRIDS[c]
        a, (sb1, sb2) = STRAYS[0] if c < 4 else STRAYS[1]
        blk_i = c % 4
        for b in range(4):
            rows = slice(STRIP * c + BLK * b, STRIP * c + BLK * (b + 1))
            for col in ACOL[b]:
                Eall[rows] += ro[:, col]
            Ep[rows] += ro[:, 8 + 2 * b]
            if t1 == (c ^ 1):
                Ep[rows] += ro[:, 9 + 2 * b]
        srows = slice(STRIP * a + BLK * blk_i, STRIP * a + BLK * (blk_i + 1))
        Eall[srows] += ro[:, 5]
        for g, tg_ in enumerate((t1, t2, t3)):
            rows = slice(STRIP * tg_, STRIP * (tg_ + 1))
            Eall[rows] += cs[g]
            if tg_ == (c ^ 1):
                Ep[rows] += cs[g]
        Eall[STRIP * sb1:STRIP * (sb1 + 1)] += cs[3]
        Eall[STRIP * sb2:STRIP * (sb2 + 1)] += cs[4]

    qsum = np.stack([Xb[part_s == p].sum(axis=0) for p in range(P)])
    onehot = np.zeros((N, C))
    onehot[np.arange(N), t_s] = 1.0
    clssum = onehot.T @ Xb
    cpsum = np.stack([onehot[part_s == p].T @ Xb[part_s == p] for p in range(P)])
    Ec = np.zeros(N)
    Ecp = np.zeros(N)
    for cl in range(C):
        rows_c = np.nonzero(t_s == cl)[0]
        if len(rows_c) == 0:
            continue
        V = Xb[rows_c]
        E = np.exp(V @ V.T)
        Ec[rows_c] = E.sum(axis=1)
        pc_ = part_s[rows_c]
        for p in range(P):
            m = pc_ == p
            if m.any():
                Ecp[rows_c[m]] = E[np.ix_(m, m)].sum(axis=1)

    Pq = (Xb * qsum[part_s]).sum(axis=1)
    Mp = (Xb * clssum[t_s]).sum(axis=1)
    Mpq = (Xb * cpsum[part_s, t_s]).sum(axis=1)
    cnt_c = 4.0 * bc[t_s]
    cnt_cp = 1.0 * bc[t_s]

    S = Eall - Ep - Ec + Ecp
    Ls = np.log(S)
    Gp = 1024.0 * Ls - Pq + Ep / S
    Gc = cnt_c * Ls - Mp + Ec / S
    Gcp = cnt_cp * Ls - Mpq + Ecp / S
    total = float((2.0 * Gp + Gc - 3.0 * Gcp).sum())
    return np.float32(total / N)


def kernel(inputs, targets):
    if "nc" not in _CACHE:
        _CACHE["nc"] = _build_nc()
    nc = _CACHE["nc"]
    in_maps, aux = host_prep(inputs, targets)
    res = bass_utils.run_bass_kernel_spmd(
        nc, in_maps, core_ids=list(range(NCORES)))
    _CACHE["last_results"] = res
    outs = [{"ro": r["ro"], "cs": r["cs"]} for r in res.results]
    return host_combine(outs, aux)


# revision 10
# speedup vs baseline: 1.6530x; 1.0302x over previous
"""Trainium2 Bass kernel for nn_NPairsLoss — symmetric half-Gram, v3.

See kernel_v2 docstring for the math/cover.  v3 engineering changes:
- 4 input DMAs with 3D access patterns (HWDGE fixed cost is 625ns/DMA,
  serialized — v2's 30 piece DMAs burned 19us of HWDGE).
- consts via Pool memset (no DMA); early 1-element Exp warms the ACT table
  off the critical path.
- vcol slots [L4 | R1 R2 R3 | R5 R6 | H]; main psum chunk order
  [diag | R1 | R2 | R3] so block 0 / block 3 tiles can split into
  diag-only + R-only activations (earlier start, shorter tail).
- colsums: DVE fp16 accumulate for blocks 0-2, PE ones-matmuls directly on
  block 3's and the stray's E tiles (PSUM-accumulated), halves split between
  ACT and DVE for the PSUM->SBUF copy.
"""
import os
from contextlib import ExitStack

import numpy as np
import ml_dtypes

import concourse.bass as bass
import concourse.tile as tile
from concourse import bacc, mybir
from concourse import bass_utils

B, P, D, C = 1024, 4, 512, 128
N = B * P
NCORES = 8
STRIP = 512
BLK = 128
VC = 3200   # L4[0:512) R1[512:1024) R2[1024:1536) R3[1536:2048) R5 R6 H[3072:3200)
FP8 = mybir.dt.float8e4
F16 = mybir.dt.float16
F32 = mybir.dt.float32
nfp8 = ml_dtypes.float8_e4m3fn

STRAYS = [(0, (2, 4)), (1, (3, 5))]
STRAY_PAIRS = {(0, 2), (0, 4), (1, 3), (1, 5)}

_CACHE = {}


def _solve_cover():
    edges = [(i, j) for i in range(8) for j in range(i + 1, 8)
             if (i, j) not in STRAY_PAIRS]
    assert len(edges) == 24
    out = {c: [] for c in range(8)}

    def bt(k):
        if k == len(edges):
            return True
        i, j = edges[k]
        for c, t in ((i, j), (j, i)):
            if len(out[c]) < 3:
                out[c].append(t)
                if bt(k + 1):
                    return True
                out[c].pop()
        return False

    assert bt(0)
    grids = {}
    for c in range(8):
        ts = out[c][:]
        p = c ^ 1
        if p in ts:
            ts.remove(p)
            ts = [p] + ts
        grids[c] = ts
    return grids


GRIDS = _solve_cover()

# colsum chunk -> (partition, col) slot in the pc PSUM tile
CSPOS = [(0, 0), (32, 0), (64, 0), (0, 512), (32, 512)]   # R1 R2 R3 R5 R6


def _build_nc():
    nc = bacc.Bacc(
        "TRN2",
        target_bir_lowering=False,
        debug=False,
        enable_asserts=False,
        num_devices=NCORES,
    )
    xt_d = nc.dram_tensor("xt", [BLK, 4 * VC], FP8, kind="ExternalInput")
    ro_d = nc.dram_tensor("ro", [BLK, 16], F32, kind="ExternalOutput")
    cs_d = nc.dram_tensor("cs", [65, 1024], F32, kind="ExternalOutput")

    AF = mybir.ActivationFunctionType
    DR = mybir.MatmulPerfMode.DoubleRow
    ADD = mybir.AluOpType.add
    AX = mybir.AxisListType.X

    with tile.TileContext(nc) as tc, ExitStack() as ctx:
        const = ctx.enter_context(tc.tile_pool(name="const", bufs=1))
        psum = ctx.enter_context(tc.tile_pool(name="psum", bufs=1, space="PSUM"))
        epool = ctx.enter_context(tc.tile_pool(name="e", bufs=2))

        zc_t = const.tile([BLK, 1], F32, tag="zc", name="zc_t")
        nc.gpsimd.memset(zc_t[:], 0.0)
        on_t = const.tile([BLK, 1], F16, tag="on", name="on_t")
        nc.gpsimd.memset(on_t[:], 1.0)
        warm = const.tile([BLK, 1], F16, tag="warm", name="warm")
        nc.scalar.activation(warm[:], zc_t[:], AF.Exp, bias=zc_t[:])

        xt = const.tile([BLK, 4 * VC], FP8, tag="xt", name="xt")
        xt3 = xt[:].rearrange("p (s c) -> p s c", s=4)
        xd3 = xt_d.ap().rearrange("p (s c) -> p s c", s=4)
        for lo, hi in ((0, 512), (512, 1024), (1024, 1536), (1536, 2048),
                       (2048, 3200)):
            nc.sync.dma_start(xt3[:, :, lo:hi], xd3[:, :, lo:hi])

        out_t = const.tile([BLK, 16], F32, tag="out", name="out_t")
        acc = const.tile([BLK, 1536], F16, tag="acc", name="acc")
        csb = const.tile([BLK, 1024], F32, tag="csb", name="csb")

        def main_mms(ps, b, gs=(0, 1, 2, 3)):
            lhs_lo = BLK * b
            for g in gs:             # g=0 diag (L4), g>=1 -> R_g
                rhs_lo = 0 if g == 0 else 512 * g
                for sp in range(2):
                    nc.tensor.matmul(
                        ps[:, 512 * g:512 * (g + 1)],
                        xt3[:, 2 * sp:2 * sp + 2, lhs_lo:lhs_lo + BLK],
                        xt3[:, 2 * sp:2 * sp + 2, rhs_lo:rhs_lo + 512],
                        start=(sp == 0), stop=(sp == 1), perf_mode=DR,
                    )

        def act(e_t, ps, lo, hi, col):
            nc.scalar.activation(e_t[:, lo:hi], ps[:, lo:hi], AF.Exp,
                                 bias=zc_t[:], accum_out=out_t[:, col:col + 1])

        def reduce2(e_t, b):
            nc.vector.tensor_reduce(out_t[:, 8 + 2 * b:9 + 2 * b],
                                    e_t[:, 0:512], axis=AX, op=ADD)
            nc.vector.tensor_reduce(out_t[:, 9 + 2 * b:10 + 2 * b],
                                    e_t[:, 512:1024], axis=AX, op=ADD)

        # blocks 0,1: diag chunk exp'd as soon as its matmuls land (only
        # needs the first DMA piece), R chunks follow per-piece
        e_tiles = {}
        ps01 = []
        for b in range(2):
            ps = psum.tile([BLK, 2048], F32, tag="m", bufs=2, name=f"ps{b}")
            ps01.append(ps)
            e_t = epool.tile([BLK, 2048], F16, tag="e", bufs=2, name=f"e{b}")
            e_tiles[b] = e_t
            main_mms(ps, b, gs=(0,))
            act(e_t, ps, 0, 512, 2 * b)            # d0 / d1
        for b in range(2):
            main_mms(ps01[b], b, gs=(1, 2, 3))
            act(e_tiles[b], ps01[b], 512, 2048, 2 * b + 1)   # R0 / R1
            if b == 0:
                nc.vector.tensor_copy(acc[:], e_tiles[0][:, 512:2048])
            else:
                nc.vector.tensor_add(acc[:], acc[:], e_tiles[1][:, 512:2048])
            reduce2(e_tiles[b], b)

        # block 2: full tile
        ps2 = psum.tile([BLK, 2048], F32, tag="m", bufs=2, name="ps2")
        main_mms(ps2, 2)
        e2 = epool.tile([BLK, 2048], F16, tag="e", bufs=2, name="e2")
        e_tiles[2] = e2
        act(e2, ps2, 0, 2048, 4)                   # b2
        nc.vector.tensor_add(acc[:], acc[:], e2[:, 512:2048])
        reduce2(e2, 2)

        # stray: mms -> exp (no DVE accumulation; PE-direct colsums)
        pss = psum.tile([BLK, 2048], F32, tag="m", bufs=2, name="pss")
        for g in range(2):
            rhs_lo = 2048 + 512 * g
            for sp in range(2):
                nc.tensor.matmul(
                    pss[:, 512 * g:512 * (g + 1)],
                    xt3[:, 2 * sp:2 * sp + 2, 3072:3200],
                    xt3[:, 2 * sp:2 * sp + 2, rhs_lo:rhs_lo + 512],
                    start=(sp == 0), stop=(sp == 1), perf_mode=DR,
                )
        e_s = epool.tile([BLK, 1024], F16, tag="es", bufs=1, name="es")
        act(e_s, pss, 0, 1024, 5)              # stray

        # block 3: R3 then d3 last (d3 has no colsum dependency)
        ps3 = psum.tile([BLK, 2048], F32, tag="m", bufs=2, name="ps3")
        main_mms(ps3, 3)
        e3 = epool.tile([BLK, 2048], F16, tag="e", bufs=2, name="e3")
        e_tiles[3] = e3
        act(e3, ps3, 512, 2048, 6)             # R3
        act(e3, ps3, 0, 512, 7)                # d3

        # colsums live in the stray tile's unused upper 2 PSUM banks
        # (cols 1024:2048) so they are NOT gated on d3-act freeing a
        # rotation slot.  Chunk slots: +1024 column offset vs CSPOS.
        for g in range(3, 5):
            p0, c0 = CSPOS[g]
            nc.tensor.matmul(pss[p0:p0 + 1, 1024 + c0:1024 + c0 + 512],
                             on_t[:], e_s[:, 512 * (g - 3):512 * (g - 2)],
                             start=True, stop=True)
        nc.vector.tensor_copy(csb[0:65, 512:1024], pss[0:65, 1536:2048])
        for g in range(3):
            p0, c0 = CSPOS[g]
            nc.tensor.matmul(pss[p0:p0 + 1, 1024 + c0:1024 + c0 + 512],
                             on_t[:], acc[:, 512 * g:512 * (g + 1)],
                             start=True, stop=False)
            nc.tensor.matmul(pss[p0:p0 + 1, 1024 + c0:1024 + c0 + 512],
                             on_t[:], e3[:, 512 * (g + 1):512 * (g + 2)],
                             start=False, stop=True)
        nc.vector.tensor_copy(csb[0:65, 0:512], pss[0:65, 1024:1536])
        reduce2(e3, 3)

        nc.sync.dma_start(cs_d.ap()[:, 512:1024], csb[0:65, 512:1024])
        nc.sync.dma_start(ro_d.ap(), out_t[:])
        nc.sync.dma_start(cs_d.ap()[:, 0:512], csb[0:65, 0:512])

    nc.compile()
    return nc


def host_prep(inputs, targets):
    X = np.ascontiguousarray(np.asarray(inputs, dtype=np.float32).reshape(N, D))
    tg = np.asarray(targets).astype(np.int64)
    t = np.repeat(tg, P)
    part = np.tile(np.arange(P, dtype=np.int64), B)
    order = np.lexsort((t, part))
    X_s = X[order]
    t_s = t[order]
    X8 = X_s.astype(nfp8)
    in_maps = []
    for c in range(NCORES):
        t1, t2, t3 = GRIDS[c]
        a, (sb1, sb2) = STRAYS[0] if c < 4 else STRAYS[1]
        blk_i = c % 4
        cols = np.concatenate([
            np.arange(STRIP * c, STRIP * (c + 1)),
            np.arange(STRIP * t1, STRIP * (t1 + 1)),
            np.arange(STRIP * t2, STRIP * (t2 + 1)),
            np.arange(STRIP * t3, STRIP * (t3 + 1)),
            np.arange(STRIP * sb1, STRIP * (sb1 + 1)),
            np.arange(STRIP * sb2, STRIP * (sb2 + 1)),
            np.arange(STRIP * a + BLK * blk_i, STRIP * a + BLK * (blk_i + 1)),
        ])
        assert cols.shape[0] == VC
        xtT = X8[cols].T
        xt = np.ascontiguousarray(
            xtT.reshape(4, BLK, VC).transpose(1, 0, 2).reshape(BLK, 4 * VC))
        in_maps.append({"xt": xt})
    aux = dict(Xb=X8.astype(np.float64), t_s=t_s, tg=tg)
    return in_maps, aux


def host_combine(outs, aux):
    Xb, t_s, tg = aux["Xb"], aux["t_s"], aux["tg"]
    part_s = np.repeat(np.arange(P), B)
    bc = np.bincount(tg, minlength=C)

    Eall = np.zeros(N)
    Ep = np.zeros(N)
    # ro cols: 0 d0, 1 R0, 2 d1, 3 R1, 4 b2, 5 stray, 6 R3, 7 d3,
    #          8+2b diag-reduce, 9+2b R1-chunk-reduce
    ACOL = {0: (0, 1), 1: (2, 3), 2: (4,), 3: (6, 7)}
    for c in range(NCORES):
        ro = np.asarray(outs[c]["ro"], np.float64)
        csb = np.asarray(outs[c]["cs"], np.float64)
        cs = [csb[p0, c0:c0 + 512] for (p0, c0) in CSPOS]
        t1, t2, t3 = GRIDS[c]
        a, (sb1, sb2) = STRAYS[0] if c < 4 else STRAYS[1]
        blk_i = c % 4
        for b in range(4):
            rows = slice(STRIP * c + BLK * b, STRIP * c + BLK * (b + 1))
            for col in ACOL[b]:
                Eall[rows] += ro[:, col]
            Ep[rows] += ro[:, 8 + 2 * b]
            if t1 == (c ^ 1):
                Ep[rows] += ro[:, 9 + 2 * b]
        srows = slice(STRIP * a + BLK * blk_i, STRIP * a + BLK * (blk_i + 1))
        Eall[srows] += ro[:, 5]
        for g, tg_ in enumerate((t1, t2, t3)):
            rows = slice(STRIP * tg_, STRIP * (tg_ + 1))
            Eall[rows] += cs[g]
            if tg_ == (c ^ 1):
                Ep[rows] += cs[g]
        Eall[STRIP * sb1:STRIP * (sb1 + 1)] += cs[3]
        Eall[STRIP * sb2:STRIP * (sb2 + 1)] += cs[4]

    qsum = np.stack([Xb[part_s == p].sum(axis=0) for p in range(P)])
    onehot = np.zeros((N, C))
    onehot[np.arange(N), t_s] = 1.0
    clssum = onehot.T @ Xb
    cpsum = np.stack([onehot[part_s == p].T @ Xb[part_s == p] for p in range(P)])
    Ec = np.zeros(N)
    Ecp = np.zeros(N)
    for cl in range(C):
        rows_c = np.nonzero(t_s == cl)[0]
        if len(rows_c) == 0:
            continue
        V = Xb[rows_c]
        E = np.exp(V @ V.T)
        Ec[rows_c] = E.sum(axis=1)
        pc_ = part_s[rows_c]
        for p in range(P):
            m = pc_ == p
            if m.any():
                Ecp[rows_c[m]] = E[np.ix_(m, m)].sum(axis=1)

    Pq = (Xb * qsum[part_s]).sum(axis=1)
    Mp = (Xb * clssum[t_s]).sum(axis=1)
    Mpq = (Xb * cpsum[part_s, t_s]).sum(axis=1)
    cnt_c = 4.0 * bc[t_s]
    cnt_cp = 1.0 * bc[t_s]

    S = Eall - Ep - Ec + Ecp
    Ls = np.log(S)
    Gp = 1024.0 * Ls - Pq + Ep / S
    Gc = cnt_c * Ls - Mp + Ec / S
    Gcp = cnt_cp * Ls - Mpq + Ecp / S
    total = float((2.0 * Gp + Gc - 3.0 * Gcp).sum())
    return np.float32(total / N)


def kernel(inputs, targets):
    if "nc" not in _CACHE:
        _CACHE["nc"] = _build_nc()
    nc = _CACHE["nc"]
    in_maps, aux = host_prep(inputs, targets)
    res = bass_utils.run_bass_kernel_spmd(
        nc, in_maps, core_ids=list(range(NCORES)))
    _CACHE["last_results"] = res
    outs = [{"ro": r["ro"], "cs": r["cs"]} for r in res.results]
    return host_combine(outs, aux)


# revision 11
# speedup vs baseline: 1.6610x; 1.0048x over previous
"""Trainium2 Bass kernel for nn_NPairsLoss — symmetric half-Gram, v3.

See kernel_v2 docstring for the math/cover.  v3 engineering changes:
- 4 input DMAs with 3D access patterns (HWDGE fixed cost is 625ns/DMA,
  serialized — v2's 30 piece DMAs burned 19us of HWDGE).
- consts via Pool memset (no DMA); early 1-element Exp warms the ACT table
  off the critical path.
- vcol slots [L4 | R1 R2 R3 | R5 R6 | H]; main psum chunk order
  [diag | R1 | R2 | R3] so block 0 / block 3 tiles can split into
  diag-only + R-only activations (earlier start, shorter tail).
- colsums: DVE fp16 accumulate for blocks 0-2, PE ones-matmuls directly on
  block 3's and the stray's E tiles (PSUM-accumulated), halves split between
  ACT and DVE for the PSUM->SBUF copy.
"""
import os
from contextlib import ExitStack

import numpy as np
import ml_dtypes

import concourse.bass as bass
import concourse.tile as tile
from concourse import bacc, mybir
from concourse import bass_utils

B, P, D, C = 1024, 4, 512, 128
N = B * P
NCORES = 8
STRIP = 512
BLK = 128
VC = 3200   # L4[0:512) R1[512:1024) R2[1024:1536) R3[1536:2048) R5 R6 H[3072:3200)
FP8 = mybir.dt.float8e4
F16 = mybir.dt.float16
F32 = mybir.dt.float32
nfp8 = ml_dtypes.float8_e4m3fn

STRAYS = [(0, (2, 4)), (1, (3, 5))]
STRAY_PAIRS = {(0, 2), (0, 4), (1, 3), (1, 5)}

_CACHE = {}


def _solve_cover():
    edges = [(i, j) for i in range(8) for j in range(i + 1, 8)
             if (i, j) not in STRAY_PAIRS]
    assert len(edges) == 24
    out = {c: [] for c in range(8)}

    def bt(k):
        if k == len(edges):
            return True
        i, j = edges[k]
        for c, t in ((i, j), (j, i)):
            if len(out[c]) < 3:
                out[c].append(t)
                if bt(k + 1):
                    return True
                out[c].pop()
        return False

    assert bt(0)
    grids = {}
    for c in range(8):
        ts = out[c][:]
        p = c ^ 1
        if p in ts:
            ts.remove(p)
            ts = [p] + ts
        grids[c] = ts
    return grids


GRIDS = _solve_cover()

# colsum chunk -> (partition, col) slot in the pc PSUM tile
CSPOS = [(0, 0), (32, 0), (64, 0), (0, 512), (32, 512)]   # R1 R2 R3 R5 R6


def _build_nc():
    nc = bacc.Bacc(
        "TRN2",
        target_bir_lowering=False,
        debug=False,
        enable_asserts=False,
        num_devices=NCORES,
    )
    xt_d = nc.dram_tensor("xt", [BLK, 4 * VC], FP8, kind="ExternalInput")
    ro_d = nc.dram_tensor("ro", [BLK, 16], F32, kind="ExternalOutput")
    cs_d = nc.dram_tensor("cs", [65, 1024], F32, kind="ExternalOutput")

    AF = mybir.ActivationFunctionType
    DR = mybir.MatmulPerfMode.DoubleRow
    ADD = mybir.AluOpType.add
    AX = mybir.AxisListType.X

    with tile.TileContext(nc) as tc, ExitStack() as ctx:
        const = ctx.enter_context(tc.tile_pool(name="const", bufs=1))
        psum = ctx.enter_context(tc.tile_pool(name="psum", bufs=1, space="PSUM"))
        epool = ctx.enter_context(tc.tile_pool(name="e", bufs=2))

        zc_t = const.tile([BLK, 1], F32, tag="zc", name="zc_t")
        nc.gpsimd.memset(zc_t[:], 0.0)
        on_t = const.tile([BLK, 1], F16, tag="on", name="on_t")
        nc.gpsimd.memset(on_t[:], 1.0)
        warm = const.tile([BLK, 1], F16, tag="warm", name="warm")
        nc.scalar.activation(warm[:], zc_t[:], AF.Exp, bias=zc_t[:])

        xt = const.tile([BLK, 4 * VC], FP8, tag="xt", name="xt")
        xt3 = xt[:].rearrange("p (s c) -> p s c", s=4)
        xd3 = xt_d.ap().rearrange("p (s c) -> p s c", s=4)
        for lo, hi in ((0, 512), (512, 1024), (1024, 1536), (1536, 2048),
                       (2048, 3200)):
            nc.sync.dma_start(xt3[:, :, lo:hi], xd3[:, :, lo:hi])

        out_t = const.tile([BLK, 16], F32, tag="out", name="out_t")
        acc = const.tile([BLK, 1536], F16, tag="acc", name="acc")
        csb = const.tile([BLK, 1024], F32, tag="csb", name="csb")

        def main_mms(ps, b, gs=(0, 1, 2, 3)):
            lhs_lo = BLK * b
            for g in gs:             # g=0 diag (L4), g>=1 -> R_g
                rhs_lo = 0 if g == 0 else 512 * g
                for sp in range(2):
                    nc.tensor.matmul(
                        ps[:, 512 * g:512 * (g + 1)],
                        xt3[:, 2 * sp:2 * sp + 2, lhs_lo:lhs_lo + BLK],
                        xt3[:, 2 * sp:2 * sp + 2, rhs_lo:rhs_lo + 512],
                        start=(sp == 0), stop=(sp == 1), perf_mode=DR,
                    )

        def act(e_t, ps, lo, hi, col):
            nc.scalar.activation(e_t[:, lo:hi], ps[:, lo:hi], AF.Exp,
                                 bias=zc_t[:], accum_out=out_t[:, col:col + 1])

        def reduce2(e_t, b):
            nc.vector.tensor_reduce(out_t[:, 8 + 2 * b:9 + 2 * b],
                                    e_t[:, 0:512], axis=AX, op=ADD)
            nc.vector.tensor_reduce(out_t[:, 9 + 2 * b:10 + 2 * b],
                                    e_t[:, 512:1024], axis=AX, op=ADD)

        # blocks 0,1: diag chunk exp'd as soon as its matmuls land (only
        # needs the first DMA piece), R chunks follow per-piece
        e_tiles = {}
        ps01 = []
        for b in range(2):
            ps = psum.tile([BLK, 2048], F32, tag="m", bufs=2, name=f"ps{b}")
            ps01.append(ps)
            e_t = epool.tile([BLK, 2048], F16, tag="e", bufs=2, name=f"e{b}")
            e_tiles[b] = e_t
            main_mms(ps, b, gs=(0,))
            act(e_t, ps, 0, 512, 2 * b)            # d0 / d1
        for b in range(2):
            main_mms(ps01[b], b, gs=(1, 2, 3))
            act(e_tiles[b], ps01[b], 512, 2048, 2 * b + 1)   # R0 / R1
            if b == 0:
                nc.vector.tensor_copy(acc[:], e_tiles[0][:, 512:2048])
            else:
                nc.vector.tensor_add(acc[:], acc[:], e_tiles[1][:, 512:2048])
            reduce2(e_tiles[b], b)

        # block 2: full tile
        ps2 = psum.tile([BLK, 2048], F32, tag="m", bufs=2, name="ps2")
        main_mms(ps2, 2)
        e2 = epool.tile([BLK, 2048], F16, tag="e", bufs=2, name="e2")
        e_tiles[2] = e2
        act(e2, ps2, 0, 2048, 4)                   # b2
        nc.vector.tensor_add(acc[:], acc[:], e2[:, 512:2048])
        reduce2(e2, 2)

        # stray: mms -> exp (no DVE accumulation; PE-direct colsums)
        pss = psum.tile([BLK, 2048], F32, tag="m", bufs=2, name="pss")
        for g in range(2):
            rhs_lo = 2048 + 512 * g
            for sp in range(2):
                nc.tensor.matmul(
                    pss[:, 512 * g:512 * (g + 1)],
                    xt3[:, 2 * sp:2 * sp + 2, 3072:3200],
                    xt3[:, 2 * sp:2 * sp + 2, rhs_lo:rhs_lo + 512],
                    start=(sp == 0), stop=(sp == 1), perf_mode=DR,
                )
        e_s = epool.tile([BLK, 1024], F16, tag="es", bufs=1, name="es")
        act(e_s, pss, 0, 1024, 5)              # stray

        # block 3: R3 then d3 last (d3 has no colsum dependency)
        ps3 = psum.tile([BLK, 2048], F32, tag="m", bufs=2, name="ps3")
        main_mms(ps3, 3)
        e3 = epool.tile([BLK, 2048], F16, tag="e", bufs=2, name="e3")
        e_tiles[3] = e3
        act(e3, ps3, 512, 2048, 6)             # R3
        nc.vector.tensor_reduce(out_t[:, 15:16], e3[:, 512:1024],
                                axis=AX, op=ADD)   # b3 R1-chunk (pre-d3)
        act(e3, ps3, 0, 512, 7)                # d3

        # colsums live in the stray tile's unused upper 2 PSUM banks
        # (cols 1024:2048) so they are NOT gated on d3-act freeing a
        # rotation slot.  Chunk slots: +1024 column offset vs CSPOS.
        for g in range(3, 5):
            p0, c0 = CSPOS[g]
            nc.tensor.matmul(pss[p0:p0 + 1, 1024 + c0:1024 + c0 + 512],
                             on_t[:], e_s[:, 512 * (g - 3):512 * (g - 2)],
                             start=True, stop=True)
        nc.vector.tensor_copy(csb[0:65, 512:1024], pss[0:65, 1536:2048])
        for g in range(3):
            p0, c0 = CSPOS[g]
            nc.tensor.matmul(pss[p0:p0 + 1, 1024 + c0:1024 + c0 + 512],
                             on_t[:], acc[:, 512 * g:512 * (g + 1)],
                             start=True, stop=False)
            nc.tensor.matmul(pss[p0:p0 + 1, 1024 + c0:1024 + c0 + 512],
                             on_t[:], e3[:, 512 * (g + 1):512 * (g + 2)],
                             start=False, stop=True)
        nc.vector.tensor_copy(csb[0:65, 0:512], pss[0:65, 1024:1536])

        nc.sync.dma_start(cs_d.ap()[:, 512:1024], csb[0:65, 512:1024])
        nc.sync.dma_start(ro_d.ap(), out_t[:])
        nc.sync.dma_start(cs_d.ap()[:, 0:512], csb[0:65, 0:512])

    nc.compile()
    return nc


def host_prep(inputs, targets):
    X = np.ascontiguousarray(np.asarray(inputs, dtype=np.float32).reshape(N, D))
    tg = np.asarray(targets).astype(np.int64)
    t = np.repeat(tg, P)
    part = np.tile(np.arange(P, dtype=np.int64), B)
    order = np.lexsort((t, part))
    X_s = X[order]
    t_s = t[order]
    X8 = X_s.astype(nfp8)
    in_maps = []
    for c in range(NCORES):
        t1, t2, t3 = GRIDS[c]
        a, (sb1, sb2) = STRAYS[0] if c < 4 else STRAYS[1]
        blk_i = c % 4
        cols = np.concatenate([
            np.arange(STRIP * c, STRIP * (c + 1)),
            np.arange(STRIP * t1, STRIP * (t1 + 1)),
            np.arange(STRIP * t2, STRIP * (t2 + 1)),
            np.arange(STRIP * t3, STRIP * (t3 + 1)),
            np.arange(STRIP * sb1, STRIP * (sb1 + 1)),
            np.arange(STRIP * sb2, STRIP * (sb2 + 1)),
            np.arange(STRIP * a + BLK * blk_i, STRIP * a + BLK * (blk_i + 1)),
        ])
        assert cols.shape[0] == VC
        xtT = X8[cols].T
        xt = np.ascontiguousarray(
            xtT.reshape(4, BLK, VC).transpose(1, 0, 2).reshape(BLK, 4 * VC))
        in_maps.append({"xt": xt})
    aux = dict(Xb=X8.astype(np.float64), t_s=t_s, tg=tg)
    return in_maps, aux


def host_combine(outs, aux):
    Xb, t_s, tg = aux["Xb"], aux["t_s"], aux["tg"]
    part_s = np.repeat(np.arange(P), B)
    bc = np.bincount(tg, minlength=C)

    Eall = np.zeros(N)
    Ep = np.zeros(N)
    # ro cols: 0 d0, 1 R0, 2 d1, 3 R1, 4 b2, 5 stray, 6 R3, 7 d3,
    #          8+2b diag-reduce, 9+2b R1-chunk-reduce
    ACOL = {0: (0, 1), 1: (2, 3), 2: (4,), 3: (6, 7)}
    for c in range(NCORES):
        ro = np.asarray(outs[c]["ro"], np.float64)
        csb = np.asarray(outs[c]["cs"], np.float64)
        cs = [csb[p0, c0:c0 + 512] for (p0, c0) in CSPOS]
        t1, t2, t3 = GRIDS[c]
        a, (sb1, sb2) = STRAYS[0] if c < 4 else STRAYS[1]
        blk_i = c % 4
        for b in range(4):
            rows = slice(STRIP * c + BLK * b, STRIP * c + BLK * (b + 1))
            for col in ACOL[b]:
                Eall[rows] += ro[:, col]
            Ep[rows] += ro[:, 8 + 2 * b]
            if t1 == (c ^ 1):
                Ep[rows] += ro[:, 9 + 2 * b]
        srows = slice(STRIP * a + BLK * blk_i, STRIP * a + BLK * (blk_i + 1))
        Eall[srows] += ro[:, 5]
        for g, tg_ in enumerate((t1, t2, t3)):
            rows = slice(STRIP * tg_, STRIP * (tg_ + 1))
            Eall[rows] += cs[g]
            if tg_ == (c ^ 1):
                Ep[rows] += cs[g]
        Eall[STRIP * sb1:STRIP * (sb1 + 1)] += cs[3]
        Eall[STRIP * sb2:STRIP * (sb2 + 1)] += cs[4]

    qsum = np.stack([Xb[part_s == p].sum(axis=0) for p in range(P)])
    onehot = np.zeros((N, C))
    onehot[np.arange(N), t_s] = 1.0
    clssum = onehot.T @ Xb
    cpsum = np.stack([onehot[part_s == p].T @ Xb[part_s == p] for p in range(P)])
    Ec = np.zeros(N)
    Ecp = np.zeros(N)
    for cl in range(C):
        rows_c = np.nonzero(t_s == cl)[0]
        if len(rows_c) == 0:
            continue
        V = Xb[rows_c]
        E = np.exp(V @ V.T)
        Ec[rows_c] = E.sum(axis=1)
        pc_ = part_s[rows_c]
        for p in range(P):
            m = pc_ == p
            if m.any():
                Ecp[rows_c[m]] = E[np.ix_(m, m)].sum(axis=1)

    Pq = (Xb * qsum[part_s]).sum(axis=1)
    Mp = (Xb * clssum[t_s]).sum(axis=1)
    Mpq = (Xb * cpsum[part_s, t_s]).sum(axis=1)
    cnt_c = 4.0 * bc[t_s]
    cnt_cp = 1.0 * bc[t_s]

    S = Eall - Ep - Ec + Ecp
    Ls = np.log(S)
    Gp = 1024.0 * Ls - Pq + Ep / S
    Gc = cnt_c * Ls - Mp + Ec / S
    Gcp = cnt_cp * Ls - Mpq + Ecp / S
    total = float((2.0 * Gp + Gc - 3.0 * Gcp).sum())
    return np.float32(total / N)


def kernel(inputs, targets):
    if "nc" not in _CACHE:
        _CACHE["nc"] = _build_nc()
    nc = _CACHE["nc"]
    in_maps, aux = host_prep(inputs, targets)
    res = bass_utils.run_bass_kernel_spmd(
        nc, in_maps, core_ids=list(range(NCORES)))
    _CACHE["last_results"] = res
    outs = [{"ro": r["ro"], "cs": r["cs"]} for r in res.results]
    return host_combine(outs, aux)
